# revision 10
# baseline (speedup 1.0000x reference)
"""Trainium2 Bass kernel for nn_Cumulative_Probability_Layer.

Computes, for x [B, 1024], W_hazard [1024, 6], W_base [1024, 1] (+zero biases):
    base  = x @ W_base + b_base                      # [B, 1]
    haz   = relu(x @ W_hazard + b_hazard)            # [B, 6]
    out   = concat([base, cumsum(haz, 1) + base], 1) # [B, 7]

Strategy: pure data-parallel over 8 NeuronCores (batch sharding). Per core:
  - load x rows in natural layout (contiguous 16 KiB DMA descriptors)
  - PE transpose-mode matmuls turn [128b x 128d] blocks into [128d x 128b]
  - DVE/ACT copy PSUM->SBUF (rounding to fp32r when enabled)
  - stage A: accumulate W_cat.T @ xT over 8 K-chunks into PSUM [7, 512]
  - stage B: ACT relu(+bias) rows 1..6, copy row 0 -> SBUF [7, 512]
  - stage D: PE transpose [7,128] blocks back to [128, 7]; DVE cumsum along
    the 7-wide free axis; contiguous DMA store.
"""

import numpy as np
import concourse.bass as bass  # noqa: F401  (bass types referenced via bacc)
import concourse.mybir as mybir
from concourse import bacc
from concourse.tile import TileContext
from concourse.bass_utils import run_bass_kernel_spmd

N_CORES = 8
B_FULL = 32768
D = 1024
TO = 7                      # 1 base + 6 hazards
B_CORE = B_FULL // N_CORES  # 4096
GROUP = 512                 # batch rows per pipeline group
N_GROUPS = B_CORE // GROUP  # 8
KC = D // 128               # 8 contraction chunks

F32 = mybir.dt.float32
F32R = mybir.dt.float32r

AF = mybir.ActivationFunctionType


def build_nc(use_f32r=True, repeat=1):
    """Build the per-core Bass module (same NEFF runs SPMD on all cores).

    repeat>1 replays the whole (idempotent) pipeline for timing-by-deltas:
    exec(repeat=R) - exec(repeat=1) ≈ (R-1) * steady-state kernel time.
    """
    XD = F32R if use_f32r else F32

    nc = bacc.Bacc(None, target_bir_lowering=False)
    x = nc.dram_tensor("x", [B_CORE, D], F32, kind="ExternalInput")
    w = nc.dram_tensor("w", [D, TO], F32, kind="ExternalInput")
    b = nc.dram_tensor("b", [TO, 1], F32, kind="ExternalInput")
    out = nc.dram_tensor("out", [B_CORE, TO], F32, kind="ExternalOutput")
    ident = nc.inline_tensor(np.eye(128, dtype=np.float32), name="ident")

    with TileContext(nc) as tc:
        with (
            tc.tile_pool(name="const", bufs=1) as cpool,
            tc.tile_pool(name="xin", bufs=3) as xpool,
            tc.tile_pool(name="xt", bufs=2) as xtpool,
            tc.tile_pool(name="small", bufs=2) as spool,
            tc.tile_pool(name="psx", bufs=4, space="PSUM") as psx,
            tc.tile_pool(name="psh", bufs=2, space="PSUM") as psh,
            tc.tile_pool(name="pst", bufs=2, space="PSUM") as pst,
        ):
            # constants
            id_f32 = cpool.tile([128, 128], F32)
            nc.sync.dma_start(out=id_f32[:], in_=ident[:])
            if use_f32r:
                id_x = cpool.tile([128, 128], XD)
                nc.gpsimd.dma_start(out=id_x[:], in_=ident[:])
            else:
                id_x = id_f32
            w_sb = cpool.tile([128, KC, TO], XD)
            dma_w = nc.gpsimd if use_f32r else nc.sync
            dma_w.dma_start(
                out=w_sb[:], in_=w[:].rearrange("(j p) t -> p j t", p=128)
            )
            b_sb = cpool.tile([TO, 1], F32)
            nc.sync.dma_start(out=b_sb[:], in_=b[:])

            for g in range(N_GROUPS * repeat):
                b0 = (g % N_GROUPS) * GROUP
                # x natural: partition p holds rows b0+4p .. b0+4p+3
                x_nat = xpool.tile([128, 4, D], XD)
                dma_x = nc.gpsimd if use_f32r else nc.sync
                dma_x.dma_start(
                    out=x_nat[:],
                    in_=x[b0 : b0 + GROUP, :].rearrange("(p k) d -> p k d", k=4),
                )

                # transpose all 8 d-chunks: xt[d, j, k*128+p] = x[b0+4p+k, 128j+d]
                xt = xtpool.tile([128, KC, GROUP], XD)
                for j in range(KC):
                    pj = psx.tile([128, GROUP], XD, tag="pj")
                    for k in range(4):
                        nc.tensor.transpose(
                            pj[:, k * 128 : (k + 1) * 128],
                            x_nat[:, k, j * 128 : (j + 1) * 128],
                            id_x[:],
                        )
                    if j % 4 == 3:
                        nc.scalar.activation(xt[:, j, :], pj[:], AF.Copy)
                    else:
                        nc.vector.tensor_copy(xt[:, j, :], pj[:])

                # stage A: H^T[t, n] accumulated over K chunks
                ph = psh.tile([TO, GROUP], F32, tag="ph")
                for j in range(KC):
                    nc.tensor.matmul(
                        ph[:],
                        w_sb[:, j, :],
                        xt[:, j, :],
                        start=(j == 0),
                        stop=(j == KC - 1),
                    )

                # stage B: copy+bias all 7 rows, then relu the hazard rows.
                # Row layout is [h1..h6, base] (compute ops must start at a
                # 32-aligned partition, so hazards live at partitions 0..5).
                r = spool.tile([TO, GROUP], F32, tag="r")
                nc.scalar.activation(r[:], ph[:], AF.Identity, bias=b_sb[:])
                nc.vector.tensor_scalar_max(r[0 : TO - 1, :], r[0 : TO - 1, :], 0.0)

                # stage D: transpose back to [128, (k t)] then cumsum along t
                pt = pst.tile([128, 4, TO], F32, tag="pt")
                for k in range(4):
                    nc.tensor.transpose(
                        pt[:, k, :],
                        r[:, k * 128 : (k + 1) * 128],
                        id_f32[:TO, :TO],
                    )
                # pt free layout is [h1..h6, base]; build final [base, base+
                # cumsum(h)] while copying out of PSUM.
                ot = spool.tile([128, 4, TO], F32, tag="ot")
                nc.vector.tensor_copy(ot[:, :, 0], pt[:, :, TO - 1])
                for t in range(1, TO):
                    nc.vector.tensor_add(ot[:, :, t], ot[:, :, t - 1], pt[:, :, t - 1])

                nc.sync.dma_start(
                    out=out[b0 : b0 + GROUP, :].rearrange("(p k) t -> p k t", k=4),
                    in_=ot[:],
                )

    nc.finalize()
    return nc


_NC_CACHE = {}


def _get_nc(use_f32r=True, repeat=1):
    key = (use_f32r, repeat)
    if key not in _NC_CACHE:
        _NC_CACHE[key] = build_nc(use_f32r, repeat)
    return _NC_CACHE[key]


def kernel(
    x, W_hazard, b_hazard, W_base, b_base,
    _trace=False, _use_f32r=True, _repeat=1,
):
    x = np.ascontiguousarray(x, dtype=np.float32)
    W_cat = np.ascontiguousarray(
        np.concatenate([np.asarray(W_hazard), np.asarray(W_base)], axis=1),
        dtype=np.float32,
    )  # [1024, 7] — columns [h1..h6, base]
    b_cat = np.ascontiguousarray(
        np.concatenate(
            [np.asarray(b_hazard).reshape(-1), np.asarray(b_base).reshape(-1)]
        ).reshape(TO, 1),
        dtype=np.float32,
    )

    nc = _get_nc(_use_f32r, _repeat)
    in_maps = [
        {"x": x[i * B_CORE : (i + 1) * B_CORE], "w": W_cat, "b": b_cat}
        for i in range(N_CORES)
    ]
    res = run_bass_kernel_spmd(
        nc, in_maps, core_ids=list(range(N_CORES)), trace=_trace
    )
    out = np.concatenate([r["out"] for r in res.results], axis=0)
    if _trace:
        return out, res
    return out
